# revision 42
# baseline (speedup 1.0000x reference)
"""KNNGraph (k=16) Bass kernel for 8 NeuronCores — v3 (window-2 drain, legal ISA ops).

Input: x (4, 8192, 64) fp32. Output: (src, dst) int32 edge arrays of the
16-NN graph per batch (self included), matching jax.lax.top_k(-d2) order.

Sharding: core c handles batch c//2, query rows (c%2)*4096 ... +4096,
against all 8192 keys of that batch (query-row sharding, keys replicated).
Inputs are batch-normalized to unit RMS and fed as fp16 (64 dims + hi/lo
rows folding -|key|^2/2, contraction K=66).

Device loop (per core: 32 groups of 128 query rows x 2 key-halves of 4096):
  PE:   8 fp16 matmuls (N=512, fp32 PSUM) -> w = q.k - |k|^2/2, filling
        PSUM tiles A1 (keys [0:1024)), A2 ([1024:2048)), D ([2048:4096)).
  ACT:  evicts A1 then A2 to SBUF fp16 (two 1024-wide copies -> E); the
        two-tile split breaks the evict -> refill -> evict serial loop.
  DVE:  two mixed tensor_tensor maxes pair key t (evicted fp16) with key
        t+2048 (fp32 PSUM in D): pooled[t] = max(w[t], w[t+2048]).
        Hardware allows at most one PSUM operand per instruction and the
        gpsimd engine cannot touch PSUM at all, so this mixed pairing is
        the cheapest legal drain (1.04 ns/pair DVE + 0.83 ns/elem ACT).
  DMA:  ship the window-2 pooled array (4096 fp16 per query row) per group.

Window w of a row covers keys {h*4096 + t, h*4096 + 2048 + t} with
h = w//2048, t = w%2048. Host takes the top-24 pooled windows per row (a
window's max >= the 16th-best w, so the top-16 windows provably contain
all true top-16 keys; 24 gives slack for fp16 rounding), expands to 48
candidates, re-ranks them exactly, and emits edges. A pooled-threshold
widening check detects rows where reduced precision could have hidden a
qualifying window; those rare rows are recomputed exactly in fp64.
"""

import numpy as np

N, M, D = 4, 8192, 64
K = 16
NCORES = 8
QROWS = M // 2           # query rows per core
NG = QROWS // 128        # 32 groups of 128 rows
HALF = M // 2            # 4096 keys per PSUM generation
KDIM = 66                # contraction: 64 dims + hi/lo norm rows
NWIN_H = HALF // 2       # 2048 pooled values per half (window = 2 keys)
TOPW = 24                # candidate windows per row taken on host
EPS = 0.12               # w-space safety margin (f16 matmul + f16 pooling)

_COMPILED = {}


def _build_nc():
    import concourse.bacc as bacc
    import concourse.mybir as mybir
    import concourse.tile as tile

    nc = bacc.Bacc(None)
    f32 = mybir.dt.float32
    f16 = mybir.dt.float16
    mx = mybir.AluOpType.max

    q_d = nc.declare_dram_parameter("q", [KDIM, QROWS], f16, isOutput=False)
    kv_d = nc.declare_dram_parameter("kv", [KDIM, M], f16, isOutput=False)
    pooled_d = nc.declare_dram_parameter(
        "pooled", [NG, 128, 2 * NWIN_H], f16, isOutput=True
    )

    with tile.TileContext(nc) as tc:
        with (
            tc.tile_pool(name="singles", bufs=1) as singles,
            tc.tile_pool(name="psum", bufs=1, space="PSUM") as psum,
            tc.tile_pool(name="ev", bufs=2) as ev,
            tc.tile_pool(name="outp", bufs=2) as outp,
        ):
            # input chunks sized so the first matmuls start ~1.5us in;
            # spread across the SP and (otherwise idle) gpsimd DMA queues
            QCH = [(0, 128), (128, 2048), (2048, 4096)]
            KCH = [(0, 512), (512, 2048), (2048, 4096), (4096, 6144),
                   (6144, 8192)]
            q_t = [
                singles.tile([KDIM, b - a], f16, name=f"qt{i}")
                for i, (a, b) in enumerate(QCH)
            ]
            kv_t = [
                singles.tile([KDIM, b - a], f16, name=f"kvt{i}")
                for i, (a, b) in enumerate(KCH)
            ]
            # one tiny dummy matmul starts the PE p-state ramp clock at
            # t~300ns, so the first real matmuls (~1.5us) run at mid clock
            # and reach full clock ~1us sooner
            wl = singles.tile([66, 128], f16, name="wl")
            wr = singles.tile([66, 64], f16, name="wr")
            nc.vector.memset(wl[:], 0.0)
            nc.vector.memset(wr[:], 0.0)
            wps = psum.tile([128, 1024], f32, name="wps", tag="A1")
            nc.tensor.matmul(wps[:, 0:64], wl[:], wr[:], start=True, stop=True)

            nc.sync.dma_start(out=kv_t[0][:], in_=kv_d[:, 0:512])
            nc.gpsimd.dma_start(out=q_t[0][:], in_=q_d[:, 0:128])
            nc.sync.dma_start(out=kv_t[1][:], in_=kv_d[:, 512:2048])
            nc.gpsimd.dma_start(out=q_t[1][:], in_=q_d[:, 128:2048])
            nc.sync.dma_start(out=kv_t[2][:], in_=kv_d[:, 2048:4096])
            nc.gpsimd.dma_start(out=kv_t[3][:], in_=kv_d[:, 4096:6144])
            nc.sync.dma_start(out=kv_t[4][:], in_=kv_d[:, 6144:8192])
            nc.gpsimd.dma_start(out=q_t[2][:], in_=q_d[:, 2048:4096])

            def q_slice(g):
                r0 = g * 128
                for (a, b), qt in zip(QCH, q_t):
                    if a <= r0 and r0 + 128 <= b:
                        return qt[:, r0 - a:r0 - a + 128]
                raise AssertionError

            def kv_slice(h, m):
                c0 = h * HALF + m * 512
                for (a, b), kt in zip(KCH, kv_t):
                    if a <= c0 and c0 + 512 <= b:
                        return kt[:, c0 - a:c0 - a + 512]
                raise AssertionError

            for g in range(NG):
                lhsT = q_slice(g)
                for h in range(2):
                    A1 = psum.tile([128, 1024], f32, tag="A1")
                    A2 = psum.tile([128, 1024], f32, tag="A2")
                    D1 = psum.tile([128, 1024], f32, tag="D1")
                    D2 = psum.tile([128, 1024], f32, tag="D2")
                    pt = [A1, A2, D1, D2]
                    # emit D matmuls first: PE runs in order, and the next
                    # half-group's DVE is gated on D refill while A matmuls
                    # may legitimately wait on the previous ACT eviction
                    for m in (4, 5, 6, 7, 0, 1, 2, 3):
                        rhs = kv_slice(h, m)
                        out = pt[m // 2][:, (m % 2) * 512:(m % 2 + 1) * 512]
                        nc.tensor.matmul(out, lhsT, rhs, start=True, stop=True)
                    E = ev.tile([128, 2048], f16, tag="E")
                    if g == 0 and h == 0:
                        # first half-group: halve the evict->tt1 critical
                        # chain so the pipeline fills ~0.5us sooner
                        nc.scalar.copy(out=E[:, 0:512], in_=A1[:, 0:512])
                        nc.scalar.copy(out=E[:, 512:1024], in_=A1[:, 512:1024])
                    else:
                        nc.scalar.copy(out=E[:, 0:1024], in_=A1[:])
                    if g == 0 and h == 0:
                        nc.scalar.copy(out=E[:, 1024:1536], in_=A2[:, 0:512])
                        nc.scalar.copy(out=E[:, 1536:2048], in_=A2[:, 512:1024])
                    else:
                        nc.scalar.copy(out=E[:, 1024:2048], in_=A2[:])
                    if h == 0:
                        P = outp.tile([128, 2 * NWIN_H], f16, tag="P")
                    if g == 0 and h == 0:
                        nc.vector.tensor_tensor(
                            out=P[:, 0:512],
                            in0=D1[:, 0:512], in1=E[:, 0:512], op=mx,
                        )
                        nc.vector.tensor_tensor(
                            out=P[:, 512:1024],
                            in0=D1[:, 512:1024], in1=E[:, 512:1024], op=mx,
                        )
                    elif g == NG - 1 and h == 1:
                        # final half-group: separate single-writer tiles so
                        # tt1's output ships during tt2, leaving one short
                        # DMA after the last tensor_tensor
                        Pa = outp.tile([128, 1024], f16, tag="Pa")
                        Pb = outp.tile([128, 1024], f16, tag="Pb")
                        nc.vector.tensor_tensor(
                            out=Pa[:], in0=D1[:], in1=E[:, 0:1024], op=mx,
                        )
                        nc.sync.dma_start(
                            out=pooled_d[g][:, 2048:2560], in_=Pa[:, 0:512]
                        )
                        nc.gpsimd.dma_start(
                            out=pooled_d[g][:, 2560:3072], in_=Pa[:, 512:1024]
                        )
                        nc.vector.tensor_tensor(
                            out=Pb[:], in0=D2[:], in1=E[:, 1024:2048], op=mx,
                        )
                        # final chunks on the two fast HWDGE queues (SP and
                        # the now-idle ACT) — gpsimd's software DGE adds
                        # ~1us before the last transfer can start
                        nc.sync.dma_start(
                            out=pooled_d[g][:, 3072:3584], in_=Pb[:, 0:512]
                        )
                        nc.scalar.dma_start(
                            out=pooled_d[g][:, 3584:4096], in_=Pb[:, 512:1024]
                        )
                    else:
                        nc.vector.tensor_tensor(
                            out=P[:, h * NWIN_H:h * NWIN_H + 1024],
                            in0=D1[:], in1=E[:, 0:1024], op=mx,
                        )
                    if g == 0 and h == 0:
                        nc.vector.tensor_tensor(
                            out=P[:, 1024:1536],
                            in0=D2[:, 0:512], in1=E[:, 1024:1536], op=mx,
                        )
                        nc.vector.tensor_tensor(
                            out=P[:, 1536:2048],
                            in0=D2[:, 512:1024], in1=E[:, 1536:2048], op=mx,
                        )
                    elif not (g == NG - 1 and h == 1):
                        nc.vector.tensor_tensor(
                            out=P[:, h * NWIN_H + 1024:(h + 1) * NWIN_H],
                            in0=D2[:], in1=E[:, 1024:2048], op=mx,
                        )
                    if g == NG - 1 and h == 0:
                        # last group h0: ship halves on both queues early
                        Q = NWIN_H // 2
                        for j in range(2):
                            dq = nc.sync if j % 2 == 0 else nc.gpsimd
                            dq.dma_start(
                                out=pooled_d[g][:, j * Q:(j + 1) * Q],
                                in_=P[:, j * Q:(j + 1) * Q],
                            )
                if g < NG - 1:
                    nc.sync.dma_start(out=pooled_d[g], in_=P[:])
    if not nc.is_finalized():
        nc.finalize()
    return nc


def _win2keys():
    """(4096, 2) int32: window id -> the 2 key ids it covers."""
    w = np.arange(2 * NWIN_H)            # 4096 windows per row
    h = w // NWIN_H
    t = w % NWIN_H
    out = np.stack([h * HALF + t, h * HALF + 2048 + t], 1)
    return out.astype(np.int32)


def _prep_inputs(x):
    """Per-core input dicts. x: (N, M, D) fp32.

    Each batch is normalized to unit per-dim RMS before fp16 conversion
    (d2 ranking is scale-invariant; this keeps fp16 in range and makes
    the device's w values unit-scale). Returns (in_maps, s2) where s2[b]
    is the variance scale so host w-space thresholds can be converted.
    """
    x64 = x.astype(np.float64)
    n2 = (x64 * x64).sum(-1)                     # (N, M)
    s2 = np.maximum(n2.mean(axis=1) / D, 1e-30)  # (N,)
    xs = x64 / np.sqrt(s2)[:, None, None]
    nrm = -0.5 * (xs * xs).sum(-1)               # (N, M) scaled norms
    hi = nrm.astype(np.float16)
    lo = (nrm - hi.astype(np.float64)).astype(np.float16)
    in_maps = []
    for c in range(NCORES):
        b, h2 = c // 2, c % 2
        q = np.zeros((KDIM, QROWS), np.float16)
        q[:D] = xs[b, h2 * QROWS:(h2 + 1) * QROWS, :].T
        q[D] = 1.0
        q[D + 1] = 1.0
        kv = np.zeros((KDIM, M), np.float16)
        kv[:D] = xs[b].T
        kv[D] = hi[b]
        kv[D + 1] = lo[b]
        in_maps.append({"q": q, "kv": kv})
    return in_maps, s2


def kernel(x, k):
    x = np.asarray(x, dtype=np.float32)
    k = int(k)
    assert x.shape == (N, M, D) and k == K

    from concourse.bass_utils import run_bass_kernel_spmd

    if "nc" not in _COMPILED:
        _COMPILED["nc"] = _build_nc()
    nc = _COMPILED["nc"]

    in_maps, s2_all = _prep_inputs(x)
    res = run_bass_kernel_spmd(nc, in_maps, list(range(NCORES))).results

    NWIN = 2 * NWIN_H                            # 4096 windows per row
    pooled = np.empty((N, M, NWIN), np.float16)
    for c in range(NCORES):
        b, h2 = c // 2, c % 2
        sl = slice(h2 * QROWS, (h2 + 1) * QROWS)
        pooled[b, sl] = res[c]["pooled"].reshape(QROWS, NWIN)

    # ---- host: window selection + exact re-rank -----------------------
    import jax
    import jax.numpy as jnp

    cpu = jax.local_devices(backend="cpu")[0]
    x64 = x.astype(np.float64)
    n2_64 = (x64 * x64).sum(-1)                  # (N, M)
    w2k = _win2keys()                            # (4096, 2)

    with jax.default_device(cpu):
        topk_fn = _COMPILED.setdefault(
            "topk", jax.jit(lambda p: jax.lax.top_k(p.astype(jnp.float32), TOPW))
        )
        rerank_fn = _COMPILED.setdefault(
            "rerank",
            jax.jit(
                lambda X, n2, cand, q0: n2[cand]
                - 2.0 * jnp.einsum("rcd,rd->rc", X[cand], X[q0])
            ),
        )

        src_parts = []
        for b in range(N):
            pv, wins = topk_fn(pooled[b])        # (M, TOPW)
            wins = np.asarray(wins)
            cand = w2k[wins].reshape(M, TOPW * 2)
            n2_32 = (x[b].astype(np.float32) ** 2).sum(-1)
            sc = np.asarray(
                rerank_fn(x[b], n2_32, cand, np.arange(M, dtype=np.int32))
            )
            # order candidates by ascending index first, then stable-sort by
            # score -> ties broken by lower index, matching jax.lax.top_k
            perm = np.argsort(cand, axis=1, kind="stable")
            cand_s = np.take_along_axis(cand, perm, axis=1)
            sc_s = np.take_along_axis(sc, perm, axis=1)
            # a key can be pulled in by two windows now; push duplicate
            # occurrences out of the ranking
            dup = cand_s[:, 1:] == cand_s[:, :-1]
            sc_s[:, 1:][dup] = np.inf
            order = np.argsort(sc_s, axis=1, kind="stable")[:, : K + 1]
            top_idx = np.take_along_axis(cand_s, order[:, :K], axis=1)
            sc17 = np.take_along_axis(sc_s, order, axis=1)

            s2 = float(s2_all[b])

            # fp64 refinement of rows with near-ties anywhere in the top-17
            shaky = np.nonzero(
                (np.diff(sc17, axis=1) < 1e-3 * s2).any(axis=1)
            )[0]
            if shaky.size:
                ks = x64[b][cand_s[shaky]]                     # (s, C, 64)
                sc64 = n2_64[b][cand_s[shaky]] - 2.0 * np.einsum(
                    "rcd,rd->rc", ks, x64[b][shaky]
                )
                sc64[:, 1:][dup[shaky]] = np.inf
                o64 = np.argsort(sc64, axis=1, kind="stable")[:, :K]
                top_idx[shaky] = np.take_along_axis(cand_s[shaky], o64, axis=1)
                sc17[shaky, K - 1] = np.take_along_axis(
                    sc64, o64[:, K - 1:K], axis=1
                )[:, 0].astype(np.float32)

            # pooled-threshold widening check: a window whose pooled value
            # beats the 16th-best w minus EPS might hide a true neighbor
            w16 = -0.5 * sc17[:, K - 1].astype(np.float64) / s2
            qual = (
                pooled[b].astype(np.float32) >= (w16[:, None] - EPS)
            ).sum(1)
            redo = np.nonzero(qual > TOPW)[0]
            if redo.size:
                dots = x64[b][redo] @ x64[b].T                 # (r, M) BLAS
                d2r = n2_64[b][redo][:, None] + n2_64[b][None, :] - 2.0 * dots
                orr = np.argsort(d2r, axis=1, kind="stable")[:, :K]
                top_idx[redo] = orr

            src_parts.append(top_idx.astype(np.int64) + b * M)

    src = np.concatenate(src_parts).reshape(-1).astype(np.int32)
    dst = np.repeat(np.arange(N * M, dtype=np.int32), K)
    return src, dst


if __name__ == "__main__":
    rng = np.random.default_rng(0)
    xt = rng.standard_normal((N, M, D), dtype=np.float32)
    s, d = kernel(xt, 16)
    print(s[:32], d[:32])
